# revision 16
# baseline (speedup 1.0000x reference)
"""Axial attention (no softmax) on 8 TRN2 NeuronCores.

Problem: x (8, 64, 64, 1024) fp32; two self-attentions (16 heads, no
softmax, scale d**-0.5) along the H axis (w_qkv0/w_out0) and the W axis
(w_qkv1/w_out1); output is their sum.

Sharding: data-parallel over batch B=8 -> one batch slab per core,
weights replicated. Each core computes both axial passes for its slab;
no collectives.

Per-core kernel structure (all matmuls bf16, fp32 PSUM accumulate):
  tokens t = h*64 + w (h-major), NT = 4096 per slab.
  For each pass (H-axis then W-axis), in chunks of 8 sequences
  (CH = 512 tokens, chunk token order is sequence-major):
    1. DMA natural x tiles [128 tok, 1024], PE-transpose to
       xT [128 d, 512 tok] tiles (8 k-tiles per chunk).
    2. qkT[m] = (Wqk[:, m-block]).T @ xT  -> [128 qk-dim, 512 tok]
       (16 m-tiles, 8 k accumulation steps each; q scaled by 1/32).
    3. v[tb] = x @ Wv -> [128 tok, 1024] natural layout (4 tok-blocks).
    4. Per (head-pair j, seq-pair sp): 4-way 64x64 tile_position packs:
       A^T = kT.T @ qT   (4 matmuls into one PSUM tile)
       O^T = v.T  @ A^T  (4 matmuls into one PSUM tile)
       assembling OT[j] [128 d, 512 tok].
    5. y = OT.T @ Wout -> [128 tok, 512] fp32; pass H writes out
       directly, pass W gpsimd-DMA-accumulates (out = oh + ow).
"""

import numpy as np
import ml_dtypes
from contextlib import ExitStack

from concourse.bass_utils import run_bass_kernel_spmd
from concourse import bacc, mybir, tile
from concourse.masks import make_identity

BF16 = mybir.dt.bfloat16
F32 = mybir.dt.float32

B = 8
D = 1024
NT = 4096          # tokens per core (64*64)
CH = 512           # chunk tokens (8 sequences of 64)
NCHUNK = NT // CH  # 8
KB = D // 128      # 8 contraction blocks
SCALE = 1.0 / 32.0  # 1024 ** -0.5

_BUILD_CACHE = {}
STAGE_MAP = {}


class _TensorProxy:
    """Records which pipeline stage emitted each PE instruction (for
    trace attribution in the perf harness)."""

    def __init__(self, te):
        self._te = te
        self.stage = "?"

    def matmul(self, *a, **kw):
        r = self._te.matmul(*a, **kw)
        STAGE_MAP[r.ins.name] = self.stage
        return r

    def transpose(self, *a, **kw):
        r = self._te.transpose(*a, **kw)
        STAGE_MAP[r.ins.name] = self.stage
        return r


def build(n_chunks=NCHUNK, passes=(0, 1)):
    key = (n_chunks, tuple(passes))
    if key in _BUILD_CACHE:
        return _BUILD_CACHE[key]

    nc = bacc.Bacc("TRN2", target_bir_lowering=False, debug=False)
    xth = nc.dram_tensor("xth", [D, NT], BF16, kind="ExternalInput")
    xtw = nc.dram_tensor("xtw", [D, NT], BF16, kind="ExternalInput")
    wqk = [nc.dram_tensor(f"wqk{p}", [D, 2 * D], BF16, kind="ExternalInput")
           for p in range(2)]
    wv = [nc.dram_tensor(f"wv{p}", [D, D], BF16, kind="ExternalInput")
          for p in range(2)]
    wo = [nc.dram_tensor(f"wo{p}", [D, D], BF16, kind="ExternalInput")
          for p in range(2)]
    out = nc.dram_tensor("out", [NT, D], F32, kind="ExternalOutput")

    og = out.rearrange("(h w) d -> w h d", w=64)  # pass-H scatter view

    with tile.TileContext(nc) as tc, ExitStack() as ctx:
        def pool(name, bufs, space="SBUF"):
            return ctx.enter_context(
                tc.tile_pool(name=name, bufs=bufs, space=space))

        p_id = pool("ident", 1)
        p_wqk = pool("wqk", 16)
        p_wv = pool("wv", 8)
        p_wo = pool("wo", 8)
        p_xt = pool("xt", 24)
        p_qkt = pool("qkt", 20)
        p_v = pool("v", 8)
        p_sa = pool("sa", 18)
        p_ot = pool("ot", 16)
        p_y = pool("y", 4)
        # PSUM budget: 8 banks total (each tile is padded to one bank).
        # Row-tiled 64x64 matmul packs need the two row tiles' outputs in
        # DIFFERENT banks (concurrent row tiles may not share a bank).
        ps_big = pool("psb", 3, "PSUM")    # [128, 512] f32 qkv/y groups
        ps_att = pool("psatt", 5, "PSUM")  # transpose + A^T/O^T halves

        te = _TensorProxy(nc.tensor)
        ident = p_id.tile([128, 128], BF16, name="ident")
        make_identity(nc, ident)

        # PE warm-up: ~5us of dummy matmuls while the first DMAs land,
        # so the HAM clock gate reaches 8/8 before real work starts.
        te.stage = "warm"
        warm_ps = ps_big.tile([128, 128], F32, tag="big", name="warm_ps")
        for _ in range(16):
            te.matmul(warm_ps[:], lhsT=ident[:], rhs=ident[:],
                      start=True, stop=True)

        xt_pre = {}

        def fetch_xt(p, c, engs=(nc.sync, nc.scalar, nc.gpsimd)):
            xsrc = xth if p == 0 else xtw
            ts = []
            for k in range(KB):
                t = p_xt.tile([128, CH], BF16, tag="xt", name=f"xt_{p}_{c}_{k}")
                engs[(c * KB + k) % 3].dma_start(
                    t[:], xsrc[k * 128:(k + 1) * 128, c * CH:(c + 1) * CH])
                ts.append(t)
            return ts

        for p in passes:
            if p == passes[0]:
                # interleave chunk-0 xt and wqk tiles on 4 queues so the
                # k-th (xt, wqk) pair lands together and early: chunk-0
                # qkT accumulates k-outer, paced by these arrivals.
                q4 = (nc.sync, nc.scalar, nc.gpsimd)
                xsrc = xth if p == 0 else xtw
                ts = []
                wqk_first = []
                for k in range(KB):
                    t = p_xt.tile([128, CH], BF16, tag="xt",
                                  name=f"xt_{p}_0_{k}")
                    q4[k % 3].dma_start(
                        t[:], xsrc[k * 128:(k + 1) * 128, 0:CH])
                    ts.append(t)
                    w = p_wqk.tile([128, 2 * D], BF16, tag="wqk",
                                   name=f"wqk_{p}_{k}")
                    q4[k % 3].dma_start(w[:], wqk[p][k * 128:(k + 1) * 128, :])
                    wqk_first.append(w)
                xt_pre[(p, 0)] = ts
            if p == passes[0]:
                wqk_t = wqk_first
            else:
                wqk_t = []
                for k in range(KB):
                    t = p_wqk.tile([128, 2 * D], BF16, tag="wqk", name=f"wqk_{p}_{k}")
                    nc.sync.dma_start(t[:], wqk[p][k * 128:(k + 1) * 128, :])
                    wqk_t.append(t)
            wv_t = []
            for k in range(KB):
                t = p_wv.tile([128, D], BF16, tag="wv", name=f"wv_{p}_{k}")
                nc.scalar.dma_start(t[:], wv[p][k * 128:(k + 1) * 128, :])
                wv_t.append(t)
            wo_t = []
            for k in range(KB):
                t = p_wo.tile([128, D], BF16, tag="wo", name=f"wo_{p}_{k}")
                nc.scalar.dma_start(t[:], wo[p][k * 128:(k + 1) * 128, :])
                wo_t.append(t)

            for c in range(n_chunks):
                # 1. xT k-tiles straight from DRAM (host-pretransposed),
                # prefetched one chunk ahead.
                xt = xt_pre.pop((p, c)) if (p, c) in xt_pre else fetch_xt(p, c)
                if c + 1 < n_chunks:
                    xt_pre[(p, c + 1)] = fetch_xt(p, c + 1)
                elif p == passes[0] and len(passes) > 1:
                    xt_pre[(passes[1], 0)] = fetch_xt(passes[1], 0)

                # 2. qkT projection: 16 m-tiles, accumulate over 8 k-blocks.
                # The very first chunk runs k-outer in m-batches of 3 (the
                # ps_big depth) so compute starts with wqk[0]'s arrival
                # instead of idling (at a dropped clock) until all 8 weight
                # tiles land.
                qkt = [p_qkt.tile([128, CH], BF16, tag="qkt", name=f"qkt_{p}_{c}_{i}")
                       for i in range(16)]
                # 4. attention, batched per head-pair j: all 8 sequences'
                # A^T (and O^T) land in one PSUM bank per PE row-tile
                # (row tiles must not share a bank), 16 dense 64x64
                # matmuls per bank pair, then one copy per bank.
                # paE = head 2j (row tile 0), paO = head 2j+1 (row tile 1);
                # layout: rows (s%2)*64, cols (s//2)*64.
                te.stage = "att"
                # Software pipeline: emit A(j+1), A(j+2) between A(j) and
                # O(j) so the PSUM->SBUF copies of A(j) are fully off the
                # PE critical path.
                ot = [p_ot.tile([128, CH], BF16, tag="ot", name=f"ot_{p}_{c}_{i}") for i in range(8)]

                def emit_A(j):
                    te.stage = "attA"
                    kq = qkt[8 + j]
                    qq = qkt[j]
                    paE = ps_att.tile([128, 256], F32, tag="att", name=f"paE_{p}_{c}_{j}")
                    paO = ps_att.tile([128, 256], F32, tag="att", name=f"paO_{p}_{c}_{j}")
                    for s in range(8):
                        rp = (s % 2) * 64
                        fc = (s // 2) * 64
                        ssl = slice(s * 64, (s + 1) * 64)
                        te.matmul(
                            paE[rp:rp + 64, fc:fc + 64],
                            lhsT=kq[0:64, ssl], rhs=qq[0:64, ssl],
                            start=True, stop=True, tile_position=(0, rp))
                        te.matmul(
                            paO[rp:rp + 64, fc:fc + 64],
                            lhsT=kq[64:128, ssl], rhs=qq[64:128, ssl],
                            start=True, stop=True, tile_position=(64, rp))
                    saE = p_sa.tile([128, 256], BF16, tag="sa", name=f"saE_{p}_{c}_{j}")
                    saO = p_sa.tile([128, 256], BF16, tag="sa", name=f"saO_{p}_{c}_{j}")
                    nc.vector.tensor_copy(saE[:], paE[:])
                    nc.scalar.copy(saO[:], paO[:])
                    return saE, saO

                def emit_O(j, saE, saO):
                    te.stage = "attO"
                    poS0 = ps_att.tile([128, 256], F32, tag="att", name=f"poS0_{p}_{c}_{j}")
                    poS1 = ps_att.tile([128, 256], F32, tag="att", name=f"poS1_{p}_{c}_{j}")
                    h0 = slice((2 * j) * 64, (2 * j + 1) * 64)
                    h1 = slice((2 * j + 1) * 64, (2 * j + 2) * 64)
                    for s in range(8):
                        rv = (s % 2) * 64
                        fc = (s // 2) * 64
                        vv = v_t[s // 2]
                        dst = poS0 if s % 2 == 0 else poS1
                        te.matmul(
                            dst[0:64, fc:fc + 64],
                            lhsT=vv[rv:rv + 64, h0],
                            rhs=saE[rv:rv + 64, fc:fc + 64],
                            start=True, stop=True, tile_position=(rv, 0))
                        te.matmul(
                            dst[64:128, fc:fc + 64],
                            lhsT=vv[rv:rv + 64, h1],
                            rhs=saO[rv:rv + 64, fc:fc + 64],
                            start=True, stop=True, tile_position=(rv, 64))
                    otv = ot[j].rearrange("p (s2 par t) -> p par s2 t", par=2, t=64)
                    po0v = poS0.rearrange("p (s2 t) -> p s2 t", t=64)
                    po1v = poS1.rearrange("p (s2 t) -> p s2 t", t=64)
                    nc.vector.tensor_copy(otv[:, 0], po0v)
                    if j % 4 == 0:
                        nc.vector.tensor_copy(otv[:, 1], po1v)
                    else:
                        nc.scalar.copy(otv[:, 1], po1v)

                sa_pairs = []
                te.stage = "qkT"
                if p == passes[0] and c == 0:
                    for m0 in range(0, 16, 3):
                        ms = range(m0, min(m0 + 3, 16))
                        pqs = {m: ps_big.tile([128, CH], F32, tag="big",
                                              name=f"pq_{p}_{c}_{m}")
                               for m in ms}
                        for k in range(KB):
                            for m in ms:
                                te.matmul(
                                    pqs[m][:],
                                    lhsT=wqk_t[k][:, m * 128:(m + 1) * 128],
                                    rhs=xt[k][:],
                                    start=(k == 0), stop=(k == KB - 1))
                        for m in ms:
                            nc.vector.tensor_copy(qkt[m][:], pqs[m][:])
                else:
                    # m-pair order (j, 8+j): after pair j+1, head-pair j's
                    # q/k tiles are in SBUF, so its A pack weaves in here --
                    # one ~0.25us thin pack per ~3.5us of fat matmuls keeps
                    # the HAM clock gate at 8/8 while the pack's copies
                    # overlap the following fat work.
                    for pr in range(8):
                        for m in (pr, pr + 8):
                            pq = ps_big.tile([128, CH], F32, tag="big", name=f"pq_{p}_{c}_{m}")
                            for k in range(KB):
                                te.matmul(
                                    pq[:],
                                    lhsT=wqk_t[k][:, m * 128:(m + 1) * 128],
                                    rhs=xt[k][:],
                                    start=(k == 0), stop=(k == KB - 1))
                            nc.vector.tensor_copy(qkt[m][:], pq[:])
                        if pr >= 2:
                            te.stage = "attA"
                            sa_pairs.append(emit_A(pr - 2))
                            te.stage = "qkT"
                    for j in (6, 7):
                        sa_pairs.append(emit_A(j))

                # 3. v projection, natural [tok, d] layout
                v_t = [p_v.tile([128, D], BF16, tag="v", name=f"v_{p}_{c}_{i}") for i in range(4)]
                te.stage = "v"
                for tb in range(4):
                    for n2 in range(2):
                        pv = ps_big.tile([128, CH], F32, tag="big", name=f"pv_{p}_{c}_{tb}_{n2}")
                        for k in range(KB):
                            te.matmul(
                                pv[:],
                                lhsT=xt[k][:, tb * 128:(tb + 1) * 128],
                                rhs=wv_t[k][:, n2 * 512:(n2 + 1) * 512],
                                start=(k == 0), stop=(k == KB - 1))
                        nc.vector.tensor_copy(
                            v_t[tb][:, n2 * 512:(n2 + 1) * 512], pv[:])

                # attention: A packs were woven into the qkT stage for
                # steady-state chunks (sa_pairs filled there); the first
                # chunk emits them here. O packs run as one short block.
                te.stage = "att"
                if not sa_pairs:
                    for j in range(8):
                        sa_pairs.append(emit_A(j))
                for j in range(8):
                    emit_O(j, sa_pairs[j][0], sa_pairs[j][1])

                te.stage = "y"
                for tb in range(4):
                    ysb = p_y.tile([128, D], F32, tag="y", name=f"y_{p}_{c}_{tb}")
                    for n2 in range(2):
                        py = ps_big.tile([128, CH], F32, tag="big", name=f"py_{p}_{c}_{tb}_{n2}")
                        for i in range(KB):
                            k = (i + tb * 2 + n2) % KB
                            te.matmul(
                                py[:],
                                lhsT=ot[k][:, tb * 128:(tb + 1) * 128],
                                rhs=wo_t[k][:, n2 * 512:(n2 + 1) * 512],
                                start=(i == 0), stop=(i == KB - 1))
                        nc.vector.tensor_copy(
                            ysb[:, n2 * 512:(n2 + 1) * 512], py[:])
                    if p == 1:
                        t0 = c * CH + tb * 128
                        nc.gpsimd.dma_start(
                            out[t0:t0 + 128, :], ysb[:],
                            accum_op=mybir.AluOpType.add)
                    else:
                        w0 = c * 8 + tb * 2
                        yeng = nc.sync if tb % 2 == 0 else nc.scalar
                        yeng.dma_start(og[w0:w0 + 2, :, :], ysb[:])
    nc.compile()
    _BUILD_CACHE[key] = nc
    return nc


def _prep_inputs(x, w_qkv0, w_out0, w_qkv1, w_out1):
    bf = ml_dtypes.bfloat16
    x = np.asarray(x, dtype=np.float32)
    xth_all = np.ascontiguousarray(x.transpose(0, 3, 2, 1)
                                   .reshape(B, D, NT)).astype(bf)
    xtw_all = np.ascontiguousarray(x.transpose(0, 3, 1, 2)
                                   .reshape(B, D, NT)).astype(bf)
    common = {}
    for p, (wqkv, wout) in enumerate(((w_qkv0, w_out0), (w_qkv1, w_out1))):
        wqk_s = np.ascontiguousarray(wqkv[:, :2 * D]).copy()
        wqk_s[:, :D] *= SCALE  # fold q scale into weights (2^-5, exact)
        common[f"wqk{p}"] = wqk_s.astype(bf)
        common[f"wv{p}"] = np.ascontiguousarray(wqkv[:, 2 * D:]).astype(bf)
        common[f"wo{p}"] = np.ascontiguousarray(wout).astype(bf)
    return [{"xth": xth_all[b], "xtw": xtw_all[b], **common}
            for b in range(B)]


def kernel(x, w_qkv0, w_out0, w_qkv1, w_out1, trace=False, tmpdir=None):
    nc = build()
    in_maps = _prep_inputs(x, w_qkv0, w_out0, w_qkv1, w_out1)
    res = run_bass_kernel_spmd(nc, in_maps, core_ids=list(range(B)),
                               trace=trace, tmpdir=tmpdir)
    outs = np.stack([res.results[b]["out"] for b in range(B)])
    outs = outs.reshape(B, 64, 64, D)
    kernel.last_result = res
    return outs



# revision 18
# speedup vs baseline: 1.0853x; 1.0853x over previous
"""Axial attention (no softmax) on 8 TRN2 NeuronCores.

Problem: x (8, 64, 64, 1024) fp32; two self-attentions (16 heads, no
softmax, scale d**-0.5) along the H axis (w_qkv0/w_out0) and the W axis
(w_qkv1/w_out1); output is their sum.

Sharding: data-parallel over batch B=8 -> one batch slab per core,
weights replicated. Each core computes both axial passes for its slab;
no collectives.

Per-core kernel structure (all matmuls bf16, fp32 PSUM accumulate):
  x is pre-transposed on the HOST into two [D, NT] bf16 layouts (xth
  w-major for the H pass, xtw h-major for the W pass) -- no PE
  transposes on device. 16 chunks of 8 sequences (CH=512 tokens) are
  processed as a software pipeline; per iteration k:

    v(k) [+ A packs k-1, 4..7]    v[tb] = x @ Wv, natural layout
    qkT(k) [+ O packs k-1, 0..7]  qkT[m] = Wqk[:,m].T @ xT
    y(k-1) [+ A packs k, 0..3]    y = OT.T @ Wout, DMA out

  The 64x64 attention packs (A^T = kT.T @ qT and O^T = v.T @ A^T, 4-way
  tile_position packing) are woven one pack (~0.25us) per ~3.5us of fat
  matmuls: the PE array never idles, the HAM clock gate stays at 8/8,
  and the packs' PSUM->SBUF copies (the att stage's real cost, ~4.2MB
  per chunk on DVE+ACT) overlap fat PE work instead of serializing.

  Pass H (chunks 0-7) writes out directly; pass W (chunks 8-15)
  gpsimd-DMA-accumulates (out = oh + ow).
"""

import numpy as np
import ml_dtypes
from contextlib import ExitStack

from concourse.bass_utils import run_bass_kernel_spmd
from concourse import bacc, mybir, tile
from concourse.masks import make_identity

BF16 = mybir.dt.bfloat16
F32 = mybir.dt.float32

B = 8
D = 1024
NT = 4096
CH = 512
NCHUNK = NT // CH  # 8 per pass
KB = D // 128      # 8 contraction blocks
SCALE = 1.0 / 32.0

_BUILD_CACHE = {}
STAGE_MAP = {}


class _TensorProxy:
    def __init__(self, te):
        self._te = te
        self.stage = "?"

    def matmul(self, *a, **kw):
        r = self._te.matmul(*a, **kw)
        STAGE_MAP[r.ins.name] = self.stage
        return r


def build():
    key = "pipe"
    if key in _BUILD_CACHE:
        return _BUILD_CACHE[key]

    nc = bacc.Bacc("TRN2", target_bir_lowering=False, debug=False)
    xth = nc.dram_tensor("xth", [D, NT], BF16, kind="ExternalInput")
    xtw = nc.dram_tensor("xtw", [D, NT], BF16, kind="ExternalInput")
    wqk = [nc.dram_tensor(f"wqk{p}", [D, 2 * D], BF16, kind="ExternalInput")
           for p in range(2)]
    wv = [nc.dram_tensor(f"wv{p}", [D, D], BF16, kind="ExternalInput")
          for p in range(2)]
    wo = [nc.dram_tensor(f"wo{p}", [D, D], BF16, kind="ExternalInput")
          for p in range(2)]
    out = nc.dram_tensor("out", [NT, D], F32, kind="ExternalOutput")
    og = out.rearrange("(h w) d -> w h d", w=64)

    NIT = 2 * NCHUNK  # 16 pipeline iterations; chunk k: pass k//8, c k%8

    with tile.TileContext(nc) as tc, ExitStack() as ctx:
        def pool(name, bufs, space="SBUF"):
            return ctx.enter_context(
                tc.tile_pool(name=name, bufs=bufs, space=space))

        p_id = pool("ident", 1)
        p_wqk = pool("wqk", 12)
        p_wv = pool("wv", 8)
        p_wo = pool("wo", 10)
        p_xt = pool("xt", 16)
        p_qkt = pool("qkt", 26)
        p_v = pool("v", 12)
        p_sa = pool("sa", 36)
        p_ot = pool("ot", 16)
        p_y = pool("y", 4)
        ps_big = pool("psb", 3, "PSUM")
        ps_att = pool("psatt", 5, "PSUM")

        te = _TensorProxy(nc.tensor)
        ident = p_id.tile([128, 128], BF16, name="ident")
        make_identity(nc, ident)

        dq = (nc.sync, nc.scalar, nc.gpsimd)

        # per-chunk state, indexed by iteration number
        st = {}          # k -> dict(xt, qkt, v_t, sa, ot, ybufs)
        wts = {}         # p -> dict(wqk, wv, wo)

        def fetch_xt(k):
            p, c = k // NCHUNK, k % NCHUNK
            xsrc = xth if p == 0 else xtw
            ts = []
            for kk in range(KB):
                t = p_xt.tile([128, CH], BF16, tag="xt", name=f"xt_{k}_{kk}")
                dq[(k * KB + kk) % 3].dma_start(
                    t[:], xsrc[kk * 128:(kk + 1) * 128, c * CH:(c + 1) * CH])
                ts.append(t)
            return ts

        def fetch_weights(p, interleave_xt0=False):
            w = {"wqk": [], "wv": [], "wo": []}
            xts = []
            for k in range(KB):
                if interleave_xt0:
                    t = p_xt.tile([128, CH], BF16, tag="xt",
                                  name=f"xt_0_{k}")
                    dq[k % 3].dma_start(
                        t[:], xth[k * 128:(k + 1) * 128, 0:CH])
                    xts.append(t)
                t = p_wqk.tile([128, 2 * D], BF16, tag="wqk",
                               name=f"wqk_{p}_{k}")
                dq[k % 3].dma_start(t[:], wqk[p][k * 128:(k + 1) * 128, :])
                w["wqk"].append(t)
            for k in range(KB):
                t = p_wv.tile([128, D], BF16, tag="wv", name=f"wv_{p}_{k}")
                dq[k % 3].dma_start(t[:], wv[p][k * 128:(k + 1) * 128, :])
                w["wv"].append(t)
            for k in range(KB):
                t = p_wo.tile([128, D], BF16, tag="wo", name=f"wo_{p}_{k}")
                dq[(k + 1) % 3].dma_start(t[:], wo[p][k * 128:(k + 1) * 128, :])
                w["wo"].append(t)
            wts[p] = w
            return xts

        def cp(eng, dst, src_):
            if eng is nc.scalar:
                eng.copy(dst, src_)
            else:
                eng.tensor_copy(dst, src_)

        def emit_A(k, j):
            te.stage = "attA"
            s = st[k]
            kq = s["qkt"][8 + j]
            qq = s["qkt"][j]
            paE = ps_att.tile([128, 256], F32, tag="att", name=f"paE_{k}_{j}")
            paO = ps_att.tile([128, 256], F32, tag="att", name=f"paO_{k}_{j}")
            for sq in range(8):
                rp = (sq % 2) * 64
                fc = (sq // 2) * 64
                ssl = slice(sq * 64, (sq + 1) * 64)
                te.matmul(paE[rp:rp + 64, fc:fc + 64],
                          lhsT=kq[0:64, ssl], rhs=qq[0:64, ssl],
                          start=True, stop=True, tile_position=(0, rp))
                te.matmul(paO[rp:rp + 64, fc:fc + 64],
                          lhsT=kq[64:128, ssl], rhs=qq[64:128, ssl],
                          start=True, stop=True, tile_position=(64, rp))
            saE = p_sa.tile([128, 256], BF16, tag="sa", name=f"saE_{k}_{j}")
            saO = p_sa.tile([128, 256], BF16, tag="sa", name=f"saO_{k}_{j}")
            cp(nc.vector, saE[:], paE[:])
            cp(nc.scalar, saO[:], paO[:])
            s["sa"][j] = (saE, saO)

        def emit_O(k, j):
            te.stage = "attO"
            s = st[k]
            saE, saO = s["sa"][j]
            poS0 = ps_att.tile([128, 256], F32, tag="att", name=f"poS0_{k}_{j}")
            poS1 = ps_att.tile([128, 256], F32, tag="att", name=f"poS1_{k}_{j}")
            h0 = slice((2 * j) * 64, (2 * j + 1) * 64)
            h1 = slice((2 * j + 1) * 64, (2 * j + 2) * 64)
            for sq in range(8):
                rv = (sq % 2) * 64
                fc = (sq // 2) * 64
                vv = s["v_t"][sq // 2]
                dst = poS0 if sq % 2 == 0 else poS1
                te.matmul(dst[0:64, fc:fc + 64],
                          lhsT=vv[rv:rv + 64, h0],
                          rhs=saE[rv:rv + 64, fc:fc + 64],
                          start=True, stop=True, tile_position=(rv, 0))
                te.matmul(dst[64:128, fc:fc + 64],
                          lhsT=vv[rv:rv + 64, h1],
                          rhs=saO[rv:rv + 64, fc:fc + 64],
                          start=True, stop=True, tile_position=(rv, 64))
            otv = s["ot"][j].rearrange("p (s2 par t) -> p par s2 t",
                                       par=2, t=64)
            po0v = poS0.rearrange("p (s2 t) -> p s2 t", t=64)
            po1v = poS1.rearrange("p (s2 t) -> p s2 t", t=64)
            cp(nc.vector, otv[:, 0], po0v)
            cp(nc.scalar if j % 4 else nc.vector, otv[:, 1], po1v)

        def emit_v_stage(k, opacks):
            # v groups with previous-chunk O packs woven after every 2nd
            # group (same proven one-pack-per-3.5us density)
            p = k // NCHUNK
            s = st[k]
            wv_t = wts[p]["wv"]
            oi = 0
            for g in range(8):
                tb, n2 = g // 2, g % 2
                te.stage = "v"
                pv = ps_big.tile([128, CH], F32, tag="big",
                                 name=f"pv_{k}_{tb}_{n2}")
                for kk in range(KB):
                    te.matmul(pv[:],
                              lhsT=s["xt"][kk][:, tb * 128:(tb + 1) * 128],
                              rhs=wv_t[kk][:, n2 * 512:(n2 + 1) * 512],
                              start=(kk == 0), stop=(kk == KB - 1))
                nc.vector.tensor_copy(
                    s["v_t"][tb][:, n2 * 512:(n2 + 1) * 512], pv[:])


        def emit_qkT_stage(k, opacks, kouter=False):
            p = k // NCHUNK
            s = st[k]
            wqk_t = wts[p]["wqk"]
            te.stage = "qkT"
            if kouter:
                for m0 in range(0, 16, 3):
                    ms = range(m0, min(m0 + 3, 16))
                    pqs = {m: ps_big.tile([128, CH], F32, tag="big",
                                          name=f"pq_{k}_{m}") for m in ms}
                    for kk in range(KB):
                        for m in ms:
                            te.matmul(
                                pqs[m][:],
                                lhsT=wqk_t[kk][:, m * 128:(m + 1) * 128],
                                rhs=s["xt"][kk][:],
                                start=(kk == 0), stop=(kk == KB - 1))
                    for m in ms:
                        nc.vector.tensor_copy(s["qkt"][m][:], pqs[m][:])
                return
            # m-pair order (j, 8+j): after pair j+1, head-pair j's q/k
            # tiles are ready, so its A pack weaves in -- one ~0.25us thin
            # pack per ~3.5us of fat keeps the HAM clock gate at 8/8.
            for pr in range(8):
                for m in (pr, pr + 8):
                    pq = ps_big.tile([128, CH], F32, tag="big",
                                     name=f"pq_{k}_{m}")
                    for kk in range(KB):
                        te.matmul(pq[:],
                                  lhsT=wqk_t[kk][:, m * 128:(m + 1) * 128],
                                  rhs=s["xt"][kk][:],
                                  start=(kk == 0), stop=(kk == KB - 1))
                    nc.vector.tensor_copy(s["qkt"][m][:], pq[:])
                    te.stage = "qkT"
                if pr >= 1:
                    emit_A(k, pr - 1)
                    te.stage = "qkT"
            emit_A(k, 7)

        def emit_y_stage(k):
            p, c = k // NCHUNK, k % NCHUNK
            s = st[k]
            wo_t = wts[p]["wo"]
            for tb in range(4):
                ysb = s["ybufs"][tb]
                for n2 in range(2):
                    te.stage = "y"
                    py = ps_big.tile([128, CH], F32, tag="big",
                                     name=f"py_{k}_{tb}_{n2}")
                    for i in range(KB):
                        kk = (i + tb * 2 + n2) % KB
                        te.matmul(
                            py[:],
                            lhsT=s["ot"][kk][:, tb * 128:(tb + 1) * 128],
                            rhs=wo_t[kk][:, n2 * 512:(n2 + 1) * 512],
                            start=(i == 0), stop=(i == KB - 1))
                    nc.vector.tensor_copy(
                        ysb[:, n2 * 512:(n2 + 1) * 512], py[:])
                    if n2 == 1:
                        if p == 1:
                            t0 = c * CH + tb * 128
                            nc.gpsimd.dma_start(
                                out[t0:t0 + 128, :], ysb[:],
                                accum_op=mybir.AluOpType.add)
                        else:
                            w0 = c * 8 + tb * 2
                            yeng = nc.sync if tb % 2 == 0 else nc.scalar
                            yeng.dma_start(og[w0:w0 + 2, :, :], ysb[:])


        def new_state(k):
            st[k] = {
                "xt": None,
                "qkt": [p_qkt.tile([128, CH], BF16, tag="qkt",
                                   name=f"qkt_{k}_{i}") for i in range(16)],
                "v_t": [p_v.tile([128, D], BF16, tag="v",
                                 name=f"v_{k}_{i}") for i in range(4)],
                "sa": [None] * 8,
                "ot": [p_ot.tile([128, CH], BF16, tag="ot",
                                 name=f"ot_{k}_{i}") for i in range(8)],
                "ybufs": [p_y.tile([128, D], F32, tag="y",
                                   name=f"y_{k}_{tb}") for tb in range(4)],
            }

        # ---- prologue: weights pass 0 + chunk 0 interleaved; warmup ----
        xt0 = fetch_weights(0, interleave_xt0=True)
        te.stage = "warm"
        warm_ps = ps_big.tile([128, 128], F32, tag="big", name="warm_ps")
        for _ in range(16):
            te.matmul(warm_ps[:], lhsT=ident[:], rhs=ident[:],
                      start=True, stop=True)

        new_state(0)
        st[0]["xt"] = xt0
        xt_next = fetch_xt(1)

        for k in range(NIT):
            if k > 0:
                new_state(k)
                st[k]["xt"] = xt_next
            if k + 1 < NIT:
                xt_next = fetch_xt(k + 1)
            if k == NCHUNK - 1:
                fetch_weights(1)
            if k == 0:
                # chunk 0: k-outer qkT (starts as the weight tiles land),
                # A packs as a block after v (the k-outer can't weave)
                emit_qkT_stage(0, [], kouter=True)
                emit_v_stage(0, [])
                te.stage = "att"
                for j in range(8):
                    emit_A(0, j)
            else:
                emit_qkT_stage(k, [])
                emit_v_stage(k, [])
            if k % 2 == 1:
                # pair flush: both chunks' O packs in ONE block (one HAM
                # dip per pair instead of per chunk), then both y stages
                te.stage = "att"
                for kk in (k - 1, k):
                    for j in range(8):
                        emit_O(kk, j)
                emit_y_stage(k - 1)
                emit_y_stage(k)
                del st[k - 1]
                if k > 1:
                    pass
        del st[NIT - 1]

    nc.compile()
    _BUILD_CACHE[key] = nc
    return nc


def _prep_inputs(x, w_qkv0, w_out0, w_qkv1, w_out1):
    bf = ml_dtypes.bfloat16
    x = np.asarray(x, dtype=np.float32)
    xth_all = np.ascontiguousarray(x.transpose(0, 3, 2, 1)
                                   .reshape(B, D, NT)).astype(bf)
    xtw_all = np.ascontiguousarray(x.transpose(0, 3, 1, 2)
                                   .reshape(B, D, NT)).astype(bf)
    common = {}
    for p, (wqkv, wout) in enumerate(((w_qkv0, w_out0), (w_qkv1, w_out1))):
        wqk_s = np.ascontiguousarray(wqkv[:, :2 * D]).copy()
        wqk_s[:, :D] *= SCALE
        common[f"wqk{p}"] = wqk_s.astype(bf)
        common[f"wv{p}"] = np.ascontiguousarray(wqkv[:, 2 * D:]).astype(bf)
        common[f"wo{p}"] = np.ascontiguousarray(wout).astype(bf)
    return [{"xth": xth_all[b], "xtw": xtw_all[b], **common}
            for b in range(B)]


def kernel(x, w_qkv0, w_out0, w_qkv1, w_out1, trace=False, tmpdir=None):
    nc = build()
    in_maps = _prep_inputs(x, w_qkv0, w_out0, w_qkv1, w_out1)
    res = run_bass_kernel_spmd(nc, in_maps, core_ids=list(range(B)),
                               trace=trace, tmpdir=tmpdir)
    outs = np.stack([res.results[b]["out"] for b in range(B)])
    outs = outs.reshape(B, 64, 64, D)
    kernel.last_result = res
    return outs
